# revision 1
# baseline (speedup 1.0000x reference)
"""Local (7x7 windowed) attention Trainium2 kernel.

Problem: B=1, N=4096 (T=4, H=W=32), C=384, 8 heads x hd=48, window 7x7
zero-padded (reference semantics: padded keys score exactly 0 -> weight
exp(0), value 0).

Sharding: data-parallel over positions. 8 cores; core c owns t-slice
c//2, query rows [16*(c%2), 16*(c%2)+16) (512 queries). Each core
recomputes k/v for a 3-row halo (24 rows = 768 halo positions,
zero-padded outside the image, matching the reference's zero padding).

Device pipeline per core (all matmul operands float32r, 1 cyc/row for
N>=256):
  1. qk^T = w_qk^T @ x^T  -> (128=[q_h 64|k_h 64], 768) per head
     v    = x @ w_v       -> (768, 384), stored as [V|1] blocks (49/head)
  2. per (head, j-tile): S^T = mask_add + K^T.T @ Q^T in PSUM (banded
     i-spans), exp on ACT -> E^T
  3. out' = [V|1].T @ E^T accumulated over j-tiles -> (49+pad, 512) pairs
     row 48 = softmax denominator (+ n_oob via extra matmul)
  4. reciprocal(den) -> broadcast via matmul -> numhat = num * recip
  5. proj: out = sum_pairs numhat^T.T @ Wp_pad + bias, DMA out
"""

import os

import numpy as np

import concourse.bacc as bacc
import concourse.mybir as mybir
import concourse.tile as tile
from concourse.bass_utils import run_bass_kernel_spmd

F = mybir.dt.float32
R = mybir.dt.float32r

NH = 8
HD = 48
WIN = 7
HALF = 3
T, HH, WW = 4, 32, 32
C = 384
NPOS = T * HH * WW
SCALE = HD ** -0.5
NEG = -300.0

# per j-tile (4 halo key rows each): (i_lo, span, mask_col_offset)
SPANS = [
    (0, 128, 192),
    (0, 256, 64),
    (64, 320, 0),
    (192, 320, 0),
    (320, 192, 0),
    (448, 64, 0),
]

_CACHE = {}
LAST_RESULT = None


def _build_nc():
    if "nc" in _CACHE:
        return _CACHE["nc"]
    nc = bacc.Bacc("TRN2", target_bir_lowering=False)

    d_xT = nc.dram_tensor("xT", [128, 3, 768], R, kind="ExternalInput")
    d_wqk = nc.dram_tensor("wqk", [128, 3, 8, 128], R, kind="ExternalInput")
    d_wv = nc.dram_tensor("wv", [128, 3, 384], R, kind="ExternalInput")
    d_wp = nc.dram_tensor("wp", [128, 4, 384], R, kind="ExternalInput")
    d_bp = nc.dram_tensor("bp", [1, 384], R, kind="ExternalInput")
    d_mask = nc.dram_tensor("mask", [128, 320], R, kind="ExternalInput")
    d_noob = nc.dram_tensor("noob", [1, 512], R, kind="ExternalInput")
    d_sel2 = nc.dram_tensor("sel2", [64, 64], R, kind="ExternalInput")
    d_bsel = nc.dram_tensor("bsel", [32, 128], F, kind="ExternalInput")
    d_id = nc.dram_tensor("ident", [128, 128], R, kind="ExternalInput")
    d_ones1 = nc.dram_tensor("ones1", [1, 128], R, kind="ExternalInput")
    d_zero1 = nc.dram_tensor("zero1", [1, 128], R, kind="ExternalInput")
    d_onesD = nc.dram_tensor("onesD", [1, 32], R, kind="ExternalInput")
    d_vt = nc.dram_tensor("vt", [128, 8, 16], R, kind="ExternalInput")
    d_out = nc.dram_tensor("out", [512, 384], F, kind="ExternalOutput")

    EXP = mybir.ActivationFunctionType.Exp

    with tile.TileContext(nc) as tc:
        with tc.tile_pool(name="singles", bufs=1) as S:
            xT = S.tile([128, 3, 768], R)
            wqk = S.tile([128, 3, 8, 128], R)
            wv = S.tile([128, 3, 384], R)
            wp = S.tile([128, 4, 384], R)
            bp = S.tile([1, 384], R)
            mask = S.tile([128, 320], R)
            noob = S.tile([1, 512], R)
            sel2 = S.tile([64, 64], R)
            bsel = S.tile([32, 128], F)
            ident = S.tile([128, 128], R)
            ones1 = S.tile([1, 128], R)
            zero1 = S.tile([1, 128], R)
            onesD = S.tile([1, 32], R)
            vt = S.tile([128, 8, 16], R)
            qT2 = S.tile([128, 4, 768], R)
            kT2 = S.tile([128, 4, 768], R)
            vaug = S.tile([128, 6, 8, 64], R)
            nhat = S.tile([128, 4, 512], R)

            for sb, dr in [
                (xT, d_xT), (wqk, d_wqk), (wv, d_wv), (wp, d_wp),
                (bp, d_bp), (mask, d_mask), (noob, d_noob), (sel2, d_sel2),
                (bsel, d_bsel), (ident, d_id), (ones1, d_ones1),
                (zero1, d_zero1), (onesD, d_onesD), (vt, d_vt),
            ]:
                nc.sync.dma_start(out=sb[:], in_=dr[:])

            # ---- phase 1: q^T/k^T per head-pair, v natural -----------
            # wqk m-tile 2*pr   = [q_{2pr} | q_{2pr+1}] (cols 0:48, 64:112)
            # wqk m-tile 2*pr+1 = [k_{2pr} | k_{2pr+1}]
            with tc.tile_pool(name="psA", bufs=2, space="PSUM") as psA:
                for pr in range(4):
                    for s, dst in ((0, qT2), (1, kT2)):
                        A = psA.tile([128, 512], F, tag="A")
                        B = psA.tile([128, 256], F, tag="B")
                        for k in range(3):
                            st, sp_ = (k == 0), (k == 2)
                            nc.tensor.matmul(A[:], wqk[:, k, 2 * pr + s, :],
                                             xT[:, k, 0:512], start=st, stop=sp_)
                            nc.tensor.matmul(B[:], wqk[:, k, 2 * pr + s, :],
                                             xT[:, k, 512:768], start=st, stop=sp_)
                        nc.scalar.copy(dst[:, pr, 0:512], A[:])
                        nc.vector.tensor_copy(dst[:, pr, 512:768], B[:])
                for pt in range(6):
                    V = psA.tile([128, 384], F, tag="V")
                    for k in range(3):
                        nc.tensor.matmul(V[:], xT[:, k, 128 * pt:128 * pt + 128],
                                         wv[:, k, :], start=(k == 0), stop=(k == 2))
                    nc.vector.tensor_copy(
                        vaug[:, pt, :, 0:48],
                        V[:].rearrange("p (h d) -> p h d", h=8))
                    nc.scalar.copy(vaug[:, pt, :, 48:64], vt[:])

            # ---- phases 2-4 per head-pair ----------------------------
            with tc.tile_pool(name="psS", bufs=3, space="PSUM") as psS, \
                 tc.tile_pool(name="psO", bufs=2, space="PSUM") as psO, \
                 tc.tile_pool(name="psM", bufs=1, space="PSUM") as psM, \
                 tc.tile_pool(name="sb2", bufs=3) as sb2, \
                 tc.tile_pool(name="sbn", bufs=2) as sbn:
                for pr in range(4):
                    eTs = []
                    for e in range(2):
                        h = 2 * pr + e
                        eT = sb2.tile([128, 6, 320], R, tag="eT")
                        eTs.append(eT)
                        for jt in range(6):
                            ilo, spn, mo = SPANS[jt]
                            ps = psS.tile([128, 512], F, tag="s")
                            nc.tensor.matmul(ps[:, 0:spn], ident[:],
                                             mask[:, mo:mo + spn],
                                             start=True, stop=False)
                            nc.tensor.matmul(
                                ps[:, 0:spn],
                                kT2[64 * e:64 * e + 64, pr,
                                    128 * jt:128 * (jt + 1)],
                                qT2[64 * e:64 * e + 64, pr,
                                    96 + ilo:96 + ilo + spn],
                                start=False, stop=True)
                            nc.scalar.activation(eT[:, jt, 0:spn], ps[:, 0:spn],
                                                 EXP, scale=SCALE)
                    oTs = []
                    for e in range(2):
                        h = 2 * pr + e
                        O = psO.tile([64, 512], F, tag="O")
                        nc.tensor.matmul(O[:], zero1[:, 0:64], noob[:],
                                         start=True, stop=False,
                                         skip_group_check=True)
                        for jt in range(6):
                            ilo, spn, mo = SPANS[jt]
                            nc.tensor.matmul(
                                O[:, ilo:ilo + spn],
                                vaug[:, jt, h, :],
                                eTs[e][:, jt, 0:spn],
                                start=False, stop=(jt == 5),
                                skip_group_check=True)
                        oT = sbn.tile([64, 512], R, tag=f"oT{e}")
                        if e == 0:
                            nc.scalar.copy(oT[:], O[:])
                        else:
                            nc.vector.tensor_copy(oT[:], O[:])
                        oTs.append(oT)
                    D = psM.tile([32, 512], F, tag="D")
                    nc.tensor.matmul(D[:], sel2[:, 0:32], oTs[0][:],
                                     start=True, stop=False)
                    nc.tensor.matmul(D[:], sel2[:, 32:64], oTs[1][:],
                                     start=False, stop=False)
                    nc.tensor.matmul(D[:], onesD[:], noob[:],
                                     start=False, stop=True)
                    rec = sbn.tile([32, 512], F, tag="rec")
                    nc.vector.reciprocal(rec[:], D[:])
                    Bc0 = psM.tile([64, 512], F, tag="Bc0")
                    Bc1 = psM.tile([64, 512], F, tag="Bc1")
                    nc.tensor.matmul(Bc0[:], bsel[:, 0:64], rec[:],
                                     start=True, stop=True)
                    nc.tensor.matmul(Bc1[:], bsel[:, 64:128], rec[:],
                                     start=True, stop=True)
                    nc.vector.tensor_mul(nhat[0:64, pr, :], oTs[0][:], Bc0[:])
                    tmp1 = sbn.tile([64, 512], R, tag="tmp1")
                    nc.vector.tensor_mul(tmp1[:], oTs[1][:], Bc1[:])
                    nc.sync.dma_start(out=nhat[64:128, pr, :], in_=tmp1[:])

            # ---- phase 5: projection + bias --------------------------
            with tc.tile_pool(name="psP", bufs=2, space="PSUM") as psP, \
                 tc.tile_pool(name="sbo", bufs=2) as sbo:
                for it in range(4):
                    P = psP.tile([128, 384], F, tag="P")
                    for pr in range(4):
                        nc.tensor.matmul(P[:], nhat[:, pr, 128 * it:128 * (it + 1)],
                                         wp[:, pr, :], start=(pr == 0), stop=False)
                    nc.tensor.matmul(P[:], ones1[:], bp[:],
                                     start=False, stop=True)
                    ot = sbo.tile([128, 384], F, tag="ot")
                    nc.scalar.copy(ot[:], P[:])
                    nc.sync.dma_start(out=d_out[128 * it:128 * (it + 1), :],
                                      in_=ot[:])

    nc.compile()
    _CACHE["nc"] = nc
    return nc


def _host_consts(w_qkv, w_proj, b_proj):
    wqk = np.zeros((128, 3, 8, 128), np.float32)
    for k in range(3):
        rows = slice(k * 128, (k + 1) * 128)
        for pr in range(4):
            for s in range(2):  # 0 = q block, 1 = k block
                off = 384 * s
                wqk[:, k, 2 * pr + s, 0:48] = \
                    w_qkv[rows, off + 48 * (2 * pr):off + 48 * (2 * pr) + 48]
                wqk[:, k, 2 * pr + s, 64:112] = \
                    w_qkv[rows, off + 48 * (2 * pr + 1):off + 48 * (2 * pr + 1) + 48]
    wv = np.ascontiguousarray(
        w_qkv[:, 768:1152].reshape(3, 128, 384))
    wvp = np.ascontiguousarray(np.transpose(
        w_qkv[:, 768:1152].reshape(3, 128, 384), (1, 0, 2)))
    wp = np.zeros((128, 4, 384), np.float32)
    for pr in range(4):
        wp[0:48, pr, :] = w_proj[96 * pr:96 * pr + 48, :]
        wp[64:112, pr, :] = w_proj[96 * pr + 48:96 * pr + 96, :]
    bp = b_proj.reshape(1, 384).astype(np.float32)

    mask = np.full((128, 320), NEG, np.float32)
    for r in range(4):
        for q in range(10):
            if r <= q <= r + 6:
                blk = np.full((32, 32), NEG, np.float32)
                xj, xi = np.meshgrid(np.arange(32), np.arange(32), indexing="ij")
                blk[np.abs(xj - xi) <= 3] = 0.0
                mask[32 * r:32 * r + 32, 32 * q:32 * q + 32] = blk
    noob = np.zeros((1, 512), np.float32)
    for qy in range(16):
        for qx in range(32):
            noob[0, 32 * qy + qx] = 7.0 * (max(0, 3 - qx) + max(0, qx - 28))
    sel2 = np.zeros((64, 64), np.float32)
    sel2[48, 0] = 1.0            # D row 0 = den of even head
    sel2[48, 2:32] = 1.0         # filler rows stay finite
    sel2[48, 33] = 1.0           # D row 1 = den of odd head (B block col 33-32=1)
    bsel = np.zeros((32, 128), np.float32)
    bsel[0, 0:64] = 1.0          # Bc0 = rec row 0 broadcast
    bsel[1, 64:128] = 1.0        # Bc1 = rec row 1 broadcast
    ident = np.eye(128, dtype=np.float32)
    ones1 = np.ones((1, 128), np.float32)
    zero1 = np.zeros((1, 128), np.float32)
    onesD = np.zeros((1, 32), np.float32)
    onesD[0, 0:2] = 1.0
    vt = np.zeros((128, 8, 16), np.float32)
    vt[:, :, 0] = 1.0
    return dict(wqk=wqk, wv=wvp, wp=wp, bp=bp, mask=mask, noob=noob,
                sel2=sel2, bsel=bsel, ident=ident, ones1=ones1,
                zero1=zero1, onesD=onesD, vt=vt)


def kernel(x, w_qkv, w_proj, b_proj, H=32, W=32):
    global LAST_RESULT
    x = np.asarray(x, np.float32)
    w_qkv = np.asarray(w_qkv, np.float32)
    w_proj = np.asarray(w_proj, np.float32)
    b_proj = np.asarray(b_proj, np.float32)
    assert x.shape == (1, NPOS, C) and int(H) == 32 and int(W) == 32

    nc = _build_nc()
    consts = _host_consts(w_qkv, w_proj, b_proj)

    x4 = x[0].reshape(T, HH, WW, C)
    in_maps = []
    for c in range(8):
        t, ry0 = c // 2, 16 * (c % 2)
        xh = np.zeros((24, WW, C), np.float32)
        lo, hi = ry0 - 3, ry0 + 21
        slo, shi = max(lo, 0), min(hi, HH)
        xh[slo - lo:shi - lo] = x4[t, slo:shi]
        xT = np.ascontiguousarray(
            xh.reshape(768, C).T.reshape(3, 128, 768).transpose(1, 0, 2))
        in_maps.append({"xT": xT, **consts})

    trace = bool(int(os.environ.get("TRACE", "0")))
    res = run_bass_kernel_spmd(nc, in_maps, core_ids=list(range(8)),
                               trace=trace)
    LAST_RESULT = res
    out = np.concatenate([res.results[c]["out"] for c in range(8)], axis=0)
    return out.reshape(1, NPOS, C)



# revision 12
# speedup vs baseline: 1.3249x; 1.3249x over previous
"""Local (7x7 windowed) attention Trainium2 kernel.

Problem: B=1, N=4096 (T=4, H=W=32), C=384, 8 heads x hd=48, window 7x7
zero-padded (reference semantics: padded keys score exactly 0 -> weight
exp(0), value 0).

Sharding: data-parallel over positions. 8 cores; core c owns t-slice
c//2, query rows [16*(c%2), 16*(c%2)+16) (512 queries). Each core
recomputes k/v for a 3-row halo (24 rows = 768 halo positions,
zero-padded outside the image, matching the reference's zero padding).

Device pipeline per core (all matmul operands float32r, 1 cyc/row for
N>=256):
  1. qk^T = w_qk^T @ x^T  -> (128=[q_h 64|k_h 64], 768) per head
     v    = x @ w_v       -> (768, 384), stored as [V|1] blocks (49/head)
  2. per (head, j-tile): S^T = mask_add + K^T.T @ Q^T in PSUM (banded
     i-spans), exp on ACT -> E^T
  3. out' = [V|1].T @ E^T accumulated over j-tiles -> (49+pad, 512) pairs
     row 48 = softmax denominator (+ n_oob via extra matmul)
  4. reciprocal(den) -> broadcast via matmul -> numhat = num * recip
  5. proj: out = sum_pairs numhat^T.T @ Wp_pad + bias, DMA out
"""

import os

import ml_dtypes
import numpy as np

import concourse.bacc as bacc
import concourse.mybir as mybir
import concourse.tile as tile
from concourse.bass_utils import run_bass_kernel_spmd

F = mybir.dt.float32
R = mybir.dt.float32r
BF = mybir.dt.bfloat16
NPBF = ml_dtypes.bfloat16

NH = 8
HD = 48
WIN = 7
HALF = 3
T, HH, WW = 4, 32, 32
C = 384
NPOS = T * HH * WW
SCALE = HD ** -0.5
NEG = -300.0

# per j-tile (4 halo key rows each): (i_lo, span, mask_col_offset)
SPANS = [
    (0, 128, 192),
    (0, 256, 64),
    (64, 320, 0),
    (192, 320, 0),
    (320, 192, 0),
    (448, 64, 0),
]

_CACHE = {}
LAST_RESULT = None


def _build_nc():
    if "nc" in _CACHE:
        return _CACHE["nc"]
    nc = bacc.Bacc("TRN2", target_bir_lowering=False)

    d_xT = nc.dram_tensor("xT", [128, 3, 768], BF, kind="ExternalInput")
    d_wqk = nc.dram_tensor("wqk", [128, 3, 8, 128], BF, kind="ExternalInput")
    d_wv = nc.dram_tensor("wv", [128, 3, 384], BF, kind="ExternalInput")
    d_wp = nc.dram_tensor("wp", [128, 4, 384], BF, kind="ExternalInput")
    d_bp = nc.dram_tensor("bp", [1, 384], BF, kind="ExternalInput")
    d_mask = nc.dram_tensor("mask", [128, 320], BF, kind="ExternalInput")
    d_noob = nc.dram_tensor("noob", [1, 512], R, kind="ExternalInput")
    d_noob16 = nc.dram_tensor("noob16", [1, 512], BF, kind="ExternalInput")
    d_sel2 = nc.dram_tensor("sel2", [64, 64], R, kind="ExternalInput")
    d_bsel = nc.dram_tensor("bsel", [32, 128], R, kind="ExternalInput")
    d_id = nc.dram_tensor("ident", [128, 128], BF, kind="ExternalInput")
    d_ones1 = nc.dram_tensor("ones1", [1, 128], BF, kind="ExternalInput")
    d_zero1 = nc.dram_tensor("zero1", [1, 128], BF, kind="ExternalInput")
    d_onesD = nc.dram_tensor("onesD", [1, 32], R, kind="ExternalInput")
    d_vt = nc.dram_tensor("vt", [128, 8, 16], BF, kind="ExternalInput")
    d_out = nc.dram_tensor("out", [512, 384], F, kind="ExternalOutput")

    EXP = mybir.ActivationFunctionType.Exp

    with tile.TileContext(nc) as tc:
        with tc.tile_pool(name="singles", bufs=1) as S:
            xT = S.tile([128, 3, 768], BF)
            wqk = S.tile([128, 3, 8, 128], BF)
            wv = S.tile([128, 3, 384], BF)
            wp = S.tile([128, 4, 384], BF)
            bp = S.tile([1, 384], BF)
            mask = S.tile([128, 320], BF)
            noob = S.tile([1, 512], R)
            noob16 = S.tile([1, 512], BF)
            sel2 = S.tile([64, 64], R)
            bsel = S.tile([32, 128], R)
            ident = S.tile([128, 128], BF)
            ones1 = S.tile([1, 128], BF)
            zero1 = S.tile([1, 128], BF)
            onesD = S.tile([1, 32], R)
            vt = S.tile([128, 8, 16], BF)
            qT2 = S.tile([128, 4, 768], BF)
            kT2 = S.tile([128, 4, 768], BF)
            vaug = S.tile([128, 6, 8, 64], BF)
            nhat = S.tile([128, 4, 512], BF)

            for sb, dr in [
                (xT, d_xT), (wqk, d_wqk), (wv, d_wv), (wp, d_wp),
                (bp, d_bp), (mask, d_mask), (noob, d_noob),
                (noob16, d_noob16), (sel2, d_sel2),
                (bsel, d_bsel), (ident, d_id), (ones1, d_ones1),
                (zero1, d_zero1), (onesD, d_onesD), (vt, d_vt),
            ]:
                nc.sync.dma_start(out=sb[:], in_=dr[:])

            # ---- phase 1: q^T/k^T per head-pair, v natural -----------
            # wqk m-tile 2*pr   = [q_{2pr} | q_{2pr+1}] (cols 0:48, 64:112)
            # wqk m-tile 2*pr+1 = [k_{2pr} | k_{2pr+1}]
            with tc.tile_pool(name="psA", bufs=2, space="PSUM") as psA:
                for pr in range(4):
                    for s, dst in ((0, qT2), (1, kT2)):
                        A = psA.tile([128, 512], F, tag="A")
                        B = psA.tile([128, 256], F, tag="B")
                        for k in range(3):
                            st, sp_ = (k == 0), (k == 2)
                            nc.tensor.matmul(A[:], wqk[:, k, 2 * pr + s, :],
                                             xT[:, k, 0:512], start=st, stop=sp_)
                            nc.tensor.matmul(B[:], wqk[:, k, 2 * pr + s, :],
                                             xT[:, k, 512:768], start=st, stop=sp_)
                        nc.scalar.copy(dst[:, pr, 0:512], A[:])
                        nc.vector.tensor_copy(dst[:, pr, 512:768], B[:])
                for pt in range(6):
                    V = psA.tile([128, 384], F, tag="V")
                    for k in range(3):
                        nc.tensor.matmul(V[:], xT[:, k, 128 * pt:128 * pt + 128],
                                         wv[:, k, :], start=(k == 0), stop=(k == 2))
                    nc.vector.tensor_copy(
                        vaug[:, pt, :, 0:48],
                        V[:].rearrange("p (h d) -> p h d", h=8))
                    nc.scalar.copy(vaug[:, pt, :, 48:64], vt[:])

            # ---- phases 2-4 per head-pair ----------------------------
            with tc.tile_pool(name="psS", bufs=3, space="PSUM") as psS, \
                 tc.tile_pool(name="psO", bufs=2, space="PSUM") as psO, \
                 tc.tile_pool(name="psM", bufs=1, space="PSUM") as psM, \
                 tc.tile_pool(name="sb2", bufs=3) as sb2, \
                 tc.tile_pool(name="sbn", bufs=2) as sbn:
                for pr in range(4):
                    eTs = []
                    for e in range(2):
                        h = 2 * pr + e
                        eT = sb2.tile([128, 6, 320], BF, tag="eT")
                        eTs.append(eT)
                        for jt in range(6):
                            ilo, spn, mo = SPANS[jt]
                            ps = psS.tile([128, 512], F, tag="s")
                            nc.tensor.matmul(ps[:, 0:spn], ident[:],
                                             mask[:, mo:mo + spn],
                                             start=True, stop=False)
                            nc.tensor.matmul(
                                ps[:, 0:spn],
                                kT2[64 * e:64 * e + 64, pr,
                                    128 * jt:128 * (jt + 1)],
                                qT2[64 * e:64 * e + 64, pr,
                                    96 + ilo:96 + ilo + spn],
                                start=False, stop=True)
                            nc.scalar.activation(eT[:, jt, 0:spn], ps[:, 0:spn],
                                                 EXP, scale=SCALE)
                    oTs = []
                    for e in range(2):
                        h = 2 * pr + e
                        O = psO.tile([64, 512], F, tag="O")
                        nc.tensor.matmul(O[:], zero1[:, 0:64], noob16[:],
                                         start=True, stop=False,
                                         skip_group_check=True)
                        for jt in range(6):
                            ilo, spn, mo = SPANS[jt]
                            nc.tensor.matmul(
                                O[:, ilo:ilo + spn],
                                vaug[:, jt, h, :],
                                eTs[e][:, jt, 0:spn],
                                start=False, stop=(jt == 5),
                                skip_group_check=True)
                        oT = sbn.tile([64, 512], R, tag=f"oT{e}")
                        if e == 0:
                            nc.scalar.copy(oT[:], O[:])
                        else:
                            nc.vector.tensor_copy(oT[:], O[:])
                        oTs.append(oT)
                    D = psM.tile([32, 512], F, tag="D")
                    nc.tensor.matmul(D[:], sel2[:, 0:32], oTs[0][:],
                                     start=True, stop=False)
                    nc.tensor.matmul(D[:], sel2[:, 32:64], oTs[1][:],
                                     start=False, stop=False)
                    nc.tensor.matmul(D[:], onesD[:], noob[:],
                                     start=False, stop=True)
                    rec = sbn.tile([32, 512], R, tag="rec")
                    with nc.allow_low_precision(reason="f32r == f32 bits"):
                        nc.vector.reciprocal(rec[:], D[:])
                    Bc0 = psM.tile([64, 512], F, tag="Bc0")
                    Bc1 = psM.tile([64, 512], F, tag="Bc1")
                    nc.tensor.matmul(Bc0[:], bsel[:, 0:64], rec[:],
                                     start=True, stop=True)
                    nc.tensor.matmul(Bc1[:], bsel[:, 64:128], rec[:],
                                     start=True, stop=True)
                    nc.vector.tensor_mul(nhat[0:64, pr, :], oTs[0][:], Bc0[:])
                    tmp1 = sbn.tile([64, 512], BF, tag="tmp1")
                    nc.vector.tensor_mul(tmp1[:], oTs[1][:], Bc1[:])
                    nc.sync.dma_start(out=nhat[64:128, pr, :], in_=tmp1[:])

            # ---- phase 5: projection + bias --------------------------
            with tc.tile_pool(name="psP", bufs=2, space="PSUM") as psP, \
                 tc.tile_pool(name="sbo", bufs=2) as sbo:
                for it in range(4):
                    P = psP.tile([128, 384], F, tag="P")
                    for pr in range(4):
                        nc.tensor.matmul(P[:], nhat[:, pr, 128 * it:128 * (it + 1)],
                                         wp[:, pr, :], start=(pr == 0), stop=False)
                    nc.tensor.matmul(P[:], ones1[:], bp[:],
                                     start=False, stop=True)
                    ot = sbo.tile([128, 384], F, tag="ot")
                    nc.scalar.copy(ot[:], P[:])
                    nc.sync.dma_start(out=d_out[128 * it:128 * (it + 1), :],
                                      in_=ot[:])

    nc.compile()
    _CACHE["nc"] = nc
    return nc


def _host_consts(w_qkv, w_proj, b_proj):
    wqk = np.zeros((128, 3, 8, 128), np.float32)
    for k in range(3):
        rows = slice(k * 128, (k + 1) * 128)
        for pr in range(4):
            for s in range(2):  # 0 = q block, 1 = k block
                off = 384 * s
                wqk[:, k, 2 * pr + s, 0:48] = \
                    w_qkv[rows, off + 48 * (2 * pr):off + 48 * (2 * pr) + 48]
                wqk[:, k, 2 * pr + s, 64:112] = \
                    w_qkv[rows, off + 48 * (2 * pr + 1):off + 48 * (2 * pr + 1) + 48]
    wv = np.ascontiguousarray(
        w_qkv[:, 768:1152].reshape(3, 128, 384))
    wvp = np.ascontiguousarray(np.transpose(
        w_qkv[:, 768:1152].reshape(3, 128, 384), (1, 0, 2)))
    wp = np.zeros((128, 4, 384), np.float32)
    for pr in range(4):
        wp[0:48, pr, :] = w_proj[96 * pr:96 * pr + 48, :]
        wp[64:112, pr, :] = w_proj[96 * pr + 48:96 * pr + 96, :]
    bp = b_proj.reshape(1, 384).astype(np.float32)

    mask = np.full((128, 320), NEG, np.float32)
    for r in range(4):
        for q in range(10):
            if r <= q <= r + 6:
                blk = np.full((32, 32), NEG, np.float32)
                xj, xi = np.meshgrid(np.arange(32), np.arange(32), indexing="ij")
                blk[np.abs(xj - xi) <= 3] = 0.0
                mask[32 * r:32 * r + 32, 32 * q:32 * q + 32] = blk
    noob = np.zeros((1, 512), np.float32)
    for qy in range(16):
        for qx in range(32):
            noob[0, 32 * qy + qx] = 7.0 * (max(0, 3 - qx) + max(0, qx - 28))
    noob16 = noob.astype(NPBF)
    sel2 = np.zeros((64, 64), np.float32)
    sel2[48, 0] = 1.0            # D row 0 = den of even head
    sel2[48, 2:32] = 1.0         # filler rows stay finite
    sel2[48, 33] = 1.0           # D row 1 = den of odd head (B block col 33-32=1)
    bsel = np.zeros((32, 128), np.float32)
    bsel[0, 0:64] = 1.0          # Bc0 = rec row 0 broadcast
    bsel[1, 64:128] = 1.0        # Bc1 = rec row 1 broadcast
    ident = np.eye(128, dtype=np.float32)
    ones1 = np.ones((1, 128), np.float32)
    zero1 = np.zeros((1, 128), np.float32)
    onesD = np.zeros((1, 32), np.float32)
    onesD[0, 0:2] = 1.0
    vt = np.zeros((128, 8, 16), np.float32)
    vt[:, :, 0] = 1.0
    return dict(wqk=wqk.astype(NPBF), wv=wvp.astype(NPBF),
                wp=wp.astype(NPBF), bp=bp.astype(NPBF),
                mask=mask.astype(NPBF), noob=noob, noob16=noob16,
                sel2=sel2, bsel=bsel, ident=ident.astype(NPBF),
                ones1=ones1.astype(NPBF), zero1=zero1.astype(NPBF),
                onesD=onesD, vt=vt.astype(NPBF))


def kernel(x, w_qkv, w_proj, b_proj, H=32, W=32):
    global LAST_RESULT
    x = np.asarray(x, np.float32)
    w_qkv = np.asarray(w_qkv, np.float32)
    w_proj = np.asarray(w_proj, np.float32)
    b_proj = np.asarray(b_proj, np.float32)
    assert x.shape == (1, NPOS, C) and int(H) == 32 and int(W) == 32

    nc = _build_nc()
    consts = _host_consts(w_qkv, w_proj, b_proj)

    x4 = x[0].reshape(T, HH, WW, C)
    in_maps = []
    for c in range(8):
        t, ry0 = c // 2, 16 * (c % 2)
        xh = np.zeros((24, WW, C), np.float32)
        lo, hi = ry0 - 3, ry0 + 21
        slo, shi = max(lo, 0), min(hi, HH)
        xh[slo - lo:shi - lo] = x4[t, slo:shi]
        xT = np.ascontiguousarray(
            xh.reshape(768, C).T.reshape(3, 128, 768).transpose(1, 0, 2))
        in_maps.append({"xT": xT.astype(NPBF), **consts})

    trace = bool(int(os.environ.get("TRACE", "0")))
    res = run_bass_kernel_spmd(nc, in_maps, core_ids=list(range(8)),
                               trace=trace)
    LAST_RESULT = res
    out = np.concatenate([res.results[c]["out"] for c in range(8)], axis=0)
    return out.reshape(1, NPOS, C)



# revision 30
# speedup vs baseline: 1.7259x; 1.3027x over previous
"""Local (7x7 windowed) attention Trainium2 kernel, v2.

Problem: B=1, N=4096 (T=4, H=W=32), C=384, 8 heads x hd=48, window 7x7
zero-padded (reference semantics: padded keys score exactly 0 -> weight
exp(0), value 0).

Sharding: t x head-group. Core c owns t-slice c//2 (1024 positions, all
32 rows -- no halo) and heads [4*(c%2), 4*(c%2)+4). Each core computes
its 4 heads' attention + projection partial sum; the host adds the two
partials per t-slice (+ bias).

Device pipeline per core (bf16 matmuls, fp32 PSUM):
  1. q^T/k^T per head-pair (packed 2 heads / 128 partitions), v natural
     [pos, head, hd|1-col] for PV stationary use.
  2. per (head, jt of 4 key rows): S^T = K^T.T @ Q^T (banded query
     spans), exp on ACT, binary window mask multiply on DVE.
  3. O_pair[128, 512] per (pair, query-half): row 48/112 seeded with
     n_oob (zero-padded-key count) via init matmul, accumulates
     [V|1].T @ E^T; den lands in rows 48/112.
  4. den rows gathered to one PSUM tile by 4 sel-matmuls (32-aligned
     slots), one reciprocal_approx_fast, 4 broadcast matmuls, DVE mult.
  5. proj partial: P = sum_pr nhat_pr^T.T @ Wp_pr, bf16 out DMA.
"""

import os

import ml_dtypes
import numpy as np

import concourse.bacc as bacc
import concourse.mybir as mybir
import concourse.tile as tile
from concourse.bass_utils import run_bass_kernel_spmd

F = mybir.dt.float32
R = mybir.dt.float32r
BF = mybir.dt.bfloat16
NPBF = ml_dtypes.bfloat16

NH = 8
HD = 48
T, HH, WW = 4, 32, 32
C = 384
NPOS = T * HH * WW
NQ = HH * WW  # queries per core (one t-slice)
SCALE = HD ** -0.5

# per j-tile (4 key rows = 128 keys): (q_lo, q_hi, mask_col_offset)
SPANS8 = [
    (0, 224, 96),
    (32, 352, 0),
    (160, 480, 0),
    (288, 608, 0),
    (416, 736, 0),
    (544, 864, 0),
    (672, 992, 0),
    (800, 1024, 0),
]
# PV j-tile slices per query half: (jt, q_lo, q_hi)
PV_HALF = [
    [(0, 0, 224), (1, 32, 352), (2, 160, 480), (3, 288, 512), (4, 416, 512)],
    [(3, 512, 608), (4, 512, 736), (5, 544, 864), (6, 672, 992),
     (7, 800, 1024)],
]

_CACHE = {}
LAST_RESULT = None


def _build_nc():
    if "nc" in _CACHE:
        return _CACHE["nc"]
    nc = bacc.Bacc("TRN2", target_bir_lowering=False)

    d_xT = nc.dram_tensor("xT", [128, 3, 1024], BF, kind="ExternalInput")
    d_wqk = nc.dram_tensor("wqk", [128, 3, 4, 128], BF, kind="ExternalInput")
    d_wv = nc.dram_tensor("wv", [128, 3, 192], BF, kind="ExternalInput")
    d_wp = nc.dram_tensor("wp", [128, 2, 384], BF, kind="ExternalInput")
    d_mask = nc.dram_tensor("maskbin", [128, 320], BF, kind="ExternalInput")
    d_noob = nc.dram_tensor("noob16", [1, 1024], BF, kind="ExternalInput")
    d_e2 = nc.dram_tensor("e2", [1, 128], BF, kind="ExternalInput")
    d_sel = nc.dram_tensor("sel48", [128, 32], BF, kind="ExternalInput")
    d_bsel = nc.dram_tensor("bsel", [128, 128], BF, kind="ExternalInput")
    d_vt = nc.dram_tensor("vt", [128, 4, 16], BF, kind="ExternalInput")
    d_out = nc.dram_tensor("out", [1024, 384], BF, kind="ExternalOutput")

    EXP = mybir.ActivationFunctionType.Exp

    with tile.TileContext(nc) as tc:
        with tc.tile_pool(name="singles", bufs=1) as S:
            xT = S.tile([128, 3, 1024], BF)
            wqk = S.tile([128, 3, 4, 128], BF)
            wv = S.tile([128, 3, 192], BF)
            wp = S.tile([128, 2, 384], BF)
            mask = S.tile([128, 320], BF)
            noob16 = S.tile([1, 1024], BF)
            e2 = S.tile([1, 128], BF)
            sel48 = S.tile([128, 32], BF)
            bsel = S.tile([128, 128], BF)
            vt = S.tile([128, 4, 16], BF)
            qT2 = S.tile([128, 2, 1024], BF)
            kT2 = S.tile([128, 2, 1024], BF)
            vaug = S.tile([128, 8, 4, 64], BF)
            nhat = S.tile([128, 2, 1024], BF)
            recsF = [S.tile([128, 512], F, name=f"recF{i}") for i in range(2)]
            recsB = [S.tile([128, 512], BF, name=f"recB{i}")
                     for i in range(2)]
            oTs = [S.tile([128, 512], BF, name=f"oT{s}") for s in range(4)]

            for sb, dr in [
                (xT, d_xT), (wqk, d_wqk), (wv, d_wv), (wp, d_wp),
                (mask, d_mask), (noob16, d_noob), (e2, d_e2),
                (sel48, d_sel), (bsel, d_bsel), (vt, d_vt),
            ]:
                nc.sync.dma_start(out=sb[:], in_=dr[:])

            # ---- phase 1: q^T/k^T per head-pair, v natural -----------
            with tc.tile_pool(name="psA", bufs=2, space="PSUM") as psA:
                for ti in range(4):  # 2*pr + s; s: 0=q, 1=k
                    pr, s = divmod(ti, 2)
                    dst = qT2 if s == 0 else kT2
                    for hf in range(2):
                        A = psA.tile([128, 512], F, tag="A")
                        for k in range(3):
                            nc.tensor.matmul(
                                A[:], wqk[:, k, ti, :],
                                xT[:, k, 512 * hf:512 * hf + 512],
                                start=(k == 0), stop=(k == 2))
                        if (ti + hf) % 2 == 0:
                            nc.scalar.copy(
                                dst[:, pr, 512 * hf:512 * hf + 512], A[:])
                        else:
                            nc.vector.tensor_copy(
                                dst[:, pr, 512 * hf:512 * hf + 512], A[:])
                for pt in range(8):
                    V = psA.tile([128, 192], F, tag="V")
                    for k in range(3):
                        nc.tensor.matmul(
                            V[:], xT[:, k, 128 * pt:128 * pt + 128],
                            wv[:, k, :], start=(k == 0), stop=(k == 2))
                    nc.vector.tensor_copy(
                        vaug[:, pt, :, 0:48],
                        V[:].rearrange("p (h d) -> p h d", h=4))
                    nc.scalar.copy(vaug[:, pt, :, 48:64], vt[:])

            # ---- phases 2-3: scores, exp, mask, PV per head-pair -----
            with tc.tile_pool(name="psS", bufs=2, space="PSUM") as psS, \
                 tc.tile_pool(name="psO", bufs=2, space="PSUM") as psO, \
                 tc.tile_pool(name="psD", bufs=1, space="PSUM") as psD, \
                 tc.tile_pool(name="sb2", bufs=2) as sb2:
                Ds = [psD.tile([128, 512], F, tag=f"D{i}", name=f"D{i}")
                      for i in range(2)]
                for pr in range(2):
                    Os = [psO.tile([128, 512], F, tag=f"O{h}",
                                   name=f"O{h}") for h in range(2)]
                    for hf in range(2):
                        nc.tensor.matmul(Os[hf][:], e2[:],
                                         noob16[:, 512 * hf:512 * hf + 512],
                                         start=True, stop=False,
                                         skip_group_check=True)
                    for e in range(2):
                        eT = sb2.tile([128, 8, 320], BF, tag="eT")
                        for jt in range(8):
                            qlo, qhi, mo = SPANS8[jt]
                            spn = qhi - qlo
                            ps = psS.tile([128, 320], F, tag="s")
                            nc.tensor.matmul(
                                ps[:, 0:spn],
                                kT2[64 * e:64 * e + 64, pr,
                                    128 * jt:128 * jt + 128],
                                qT2[64 * e:64 * e + 64, pr, qlo:qhi],
                                start=True, stop=True)
                            nc.scalar.activation(eT[:, jt, 0:spn],
                                                 ps[:, 0:spn], EXP,
                                                 scale=SCALE)
                            nc.vector.tensor_mul(eT[:, jt, 0:spn],
                                                 eT[:, jt, 0:spn],
                                                 mask[:, mo:mo + spn])
                        for hf in range(2):
                            pv = PV_HALF[hf]
                            for i, (jt, lo, hi) in enumerate(pv):
                                last = (e == 1) and (i == len(pv) - 1)
                                elo = lo - SPANS8[jt][0]
                                nc.tensor.matmul(
                                    Os[hf][64 * e:64 * e + 64,
                                           lo - 512 * hf:hi - 512 * hf],
                                    vaug[:, jt, 2 * pr + e, :],
                                    eT[:, jt, elo:elo + hi - lo],
                                    start=False, stop=last,
                                    skip_group_check=True)
                    for hf in range(2):
                        s = 2 * pr + hf
                        off = 64 * (s % 2)
                        if hf == 0:
                            nc.scalar.copy(oTs[s][:], Os[hf][:])
                        else:
                            nc.vector.tensor_copy(oTs[s][:], Os[hf][:])
                        nc.tensor.matmul(Ds[s // 2][off:off + 32, :],
                                         sel48[:, 0:32], oTs[s][:],
                                         start=True, stop=True,
                                         skip_group_check=True)
                for s in range(4):
                    off = 64 * (s % 2)
                    nc.vector.reciprocal(
                        recsF[s // 2][off:off + 32, :],
                        Ds[s // 2][off:off + 32, :])
                    nc.vector.tensor_copy(
                        recsB[s // 2][off:off + 32, :],
                        recsF[s // 2][off:off + 32, :])

            # ---- phase 4: broadcast 1/den, normalize -----------------
            with tc.tile_pool(name="psB", bufs=2, space="PSUM") as psB:
                for s in range(4):
                    pr, hf = divmod(s, 2)
                    off = 64 * (s % 2)
                    Bc = psB.tile([128, 512], F, tag="Bc")
                    nc.tensor.matmul(Bc[:], bsel[off:off + 32, :],
                                     recsB[s // 2][off:off + 32, :],
                                     start=True, stop=True)
                    nc.vector.tensor_mul(
                        nhat[:, pr, 512 * hf:512 * hf + 512],
                        oTs[s][:], Bc[:])

            # ---- phase 5: projection partial + DMA out ---------------
            with tc.tile_pool(name="psP", bufs=2, space="PSUM") as psP, \
                 tc.tile_pool(name="sbo", bufs=2) as sbo:
                for it in range(8):
                    P = psP.tile([128, 384], F, tag="P")
                    for pr in range(2):
                        nc.tensor.matmul(
                            P[:], nhat[:, pr, 128 * it:128 * it + 128],
                            wp[:, pr, :], start=(pr == 0), stop=(pr == 1))
                    ot = sbo.tile([128, 384], BF, tag="ot")
                    nc.scalar.copy(ot[:], P[:])
                    nc.sync.dma_start(out=d_out[128 * it:128 * it + 128, :],
                                      in_=ot[:])

    nc.compile()
    _CACHE["nc"] = nc
    return nc


def _host_consts():
    if "consts" in _CACHE:
        return _CACHE["consts"]
    mask = np.zeros((128, 320), np.float32)
    for r in range(4):
        for o in range(10):
            if r <= o <= r + 6:
                kx = np.arange(32)[:, None]
                qx = np.arange(32)[None, :]
                mask[32 * r:32 * r + 32, 32 * o:32 * o + 32] = \
                    (np.abs(kx - qx) <= 3).astype(np.float32)
    noob = np.zeros((1, 1024), np.float32)
    for qy in range(32):
        for qx in range(32):
            oy = max(0, 3 - qy) + max(0, qy - 28)
            ox = max(0, 3 - qx) + max(0, qx - 28)
            noob[0, 32 * qy + qx] = 49 - (7 - oy) * (7 - ox)
    e2 = np.zeros((1, 128), np.float32)
    e2[0, 48] = 1.0
    e2[0, 112] = 1.0
    sel48 = np.zeros((128, 32), np.float32)
    sel48[48, 0] = 1.0
    sel48[112, 1] = 1.0
    sel48[48, 2:32] = 1.0  # filler rows stay finite for the reciprocal
    bsel = np.zeros((128, 128), np.float32)
    for off in (0, 64):
        bsel[off, 0:64] = 1.0
        bsel[off + 1, 64:128] = 1.0
    vt = np.zeros((128, 4, 16), np.float32)
    vt[:, :, 0] = 1.0
    consts = dict(maskbin=mask.astype(NPBF), noob16=noob.astype(NPBF),
                  e2=e2.astype(NPBF), sel48=sel48.astype(NPBF),
                  bsel=bsel.astype(NPBF), vt=vt.astype(NPBF))
    _CACHE["consts"] = consts
    return consts


def _host_weights(w_qkv, w_proj, hg):
    wqk = np.zeros((128, 3, 4, 128), np.float32)
    for k in range(3):
        rows = slice(k * 128, (k + 1) * 128)
        for pr in range(2):
            for s in range(2):
                g0 = hg * 4 + 2 * pr
                off = 384 * s
                wqk[:, k, 2 * pr + s, 0:48] = \
                    w_qkv[rows, off + 48 * g0:off + 48 * g0 + 48]
                wqk[:, k, 2 * pr + s, 64:112] = \
                    w_qkv[rows, off + 48 * (g0 + 1):off + 48 * (g0 + 1) + 48]
    wv = np.ascontiguousarray(
        w_qkv[:, 768 + 192 * hg:768 + 192 * hg + 192]
        .reshape(3, 128, 192).transpose(1, 0, 2))
    wp = np.zeros((128, 2, 384), np.float32)
    for pr in range(2):
        g0 = hg * 4 + 2 * pr
        wp[0:48, pr, :] = w_proj[48 * g0:48 * g0 + 48, :]
        wp[64:112, pr, :] = w_proj[48 * (g0 + 1):48 * (g0 + 1) + 48, :]
    return dict(wqk=wqk.astype(NPBF), wv=wv.astype(NPBF),
                wp=wp.astype(NPBF))


def kernel(x, w_qkv, w_proj, b_proj, H=32, W=32):
    global LAST_RESULT
    x = np.asarray(x, np.float32)
    w_qkv = np.asarray(w_qkv, np.float32)
    w_proj = np.asarray(w_proj, np.float32)
    b_proj = np.asarray(b_proj, np.float32)
    assert x.shape == (1, NPOS, C) and int(H) == 32 and int(W) == 32

    nc = _build_nc()
    consts = _host_consts()
    wmaps = [_host_weights(w_qkv, w_proj, hg) for hg in range(2)]

    x4 = x[0].reshape(T, NQ, C)
    in_maps = []
    for c in range(8):
        t, hg = c // 2, c % 2
        xT = np.ascontiguousarray(
            x4[t].T.reshape(3, 128, 1024).transpose(1, 0, 2))
        in_maps.append({"xT": xT.astype(NPBF), **wmaps[hg], **consts})

    trace = bool(int(os.environ.get("TRACE", "0")))
    res = run_bass_kernel_spmd(nc, in_maps, core_ids=list(range(8)),
                               trace=trace)
    LAST_RESULT = res
    outs = []
    for t in range(T):
        p0 = res.results[2 * t]["out"].astype(np.float32)
        p1 = res.results[2 * t + 1]["out"].astype(np.float32)
        outs.append(p0 + p1 + b_proj)
    return np.concatenate(outs, axis=0).reshape(1, NPOS, C)


# revision 32
# speedup vs baseline: 2.1143x; 1.2250x over previous
"""Local (7x7 windowed) attention Trainium2 kernel, v2.1.

Problem: B=1, N=4096 (T=4, H=W=32), C=384, 8 heads x hd=48, window 7x7
zero-padded (reference semantics: padded keys score exactly 0 -> weight
exp(0), value 0).

Sharding: t x head-group. Core c owns t-slice c//2 (1024 positions, all
32 rows -- no halo) and heads [4*(c%2), 4*(c%2)+4). Each core computes
its 4 heads' attention + projection partial sum; the host adds the two
partials per t-slice (+ bias).

Device pipeline per core (bf16 matmuls, fp32 PSUM):
  1. q^T/k^T per head-pair (packed 2 heads / 128 partitions), v natural
     [pos, head, hd|1-col] for PV stationary use.
  2. per (head, jt of 4 key rows): S^T = K^T.T @ Q^T (banded query
     spans), exp on ACT, binary window mask multiply on DVE.
  3. per (query-half, pair): O[128, 512] rows 48/112 seeded with n_oob
     via init matmul, accumulates [V|1].T @ E^T; den in rows 48/112.
  4. per half: dens gathered to one PSUM tile (sel matmuls, slots 0/32),
     reciprocal_approx_fast at base partition 0 (custom DVE op breaks at
     nonzero base!), bf16 cast, broadcast matmul, DVE normalize.
  5. proj partial per 128-query tile: P = sum_pr nhat_pr^T.T @ Wp_pr,
     bf16 out DMA. Half 0's phases 4-5 overlap half 1's PV on the PE.
"""

import os

import ml_dtypes
import numpy as np

import concourse.bacc as bacc
import concourse.mybir as mybir
import concourse.tile as tile
from concourse.bass_utils import run_bass_kernel_spmd

F = mybir.dt.float32
R = mybir.dt.float32r
BF = mybir.dt.bfloat16
NPBF = ml_dtypes.bfloat16

NH = 8
HD = 48
T, HH, WW = 4, 32, 32
C = 384
NPOS = T * HH * WW
SCALE = HD ** -0.5

# per j-tile (4 key rows = 128 keys): (q_lo, q_hi, mask_col_offset)
SPANS8 = [
    (0, 224, 96),
    (32, 352, 0),
    (160, 480, 0),
    (288, 608, 0),
    (416, 736, 0),
    (544, 864, 0),
    (672, 992, 0),
    (800, 1024, 0),
]
# PV j-tile slices per query half: (jt, q_lo, q_hi)
PV_HALF = [
    [(0, 0, 224), (1, 32, 352), (2, 160, 480), (3, 288, 512), (4, 416, 512)],
    [(3, 512, 608), (4, 512, 736), (5, 544, 864), (6, 672, 992),
     (7, 800, 1024)],
]
# j-tiles whose mask multiply runs on GpSimd instead of DVE (balance)
GPS_JT = ()

_CACHE = {}
LAST_RESULT = None


def _build_nc():
    if "nc" in _CACHE:
        return _CACHE["nc"]
    nc = bacc.Bacc("TRN2", target_bir_lowering=False)

    d_xT = [nc.dram_tensor(f"xT{k}", [128, 1024], BF, kind="ExternalInput")
            for k in range(3)]
    d_wqk = nc.dram_tensor("wqk", [128, 3, 4, 128], BF, kind="ExternalInput")
    d_wv = nc.dram_tensor("wv", [128, 3, 192], BF, kind="ExternalInput")
    d_wp = nc.dram_tensor("wp", [128, 2, 384], BF, kind="ExternalInput")
    d_mask = nc.dram_tensor("maskbin", [128, 320], BF, kind="ExternalInput")
    d_noob = nc.dram_tensor("noob16", [1, 1024], BF, kind="ExternalInput")
    d_e2 = nc.dram_tensor("e2", [1, 128], BF, kind="ExternalInput")
    d_sel = nc.dram_tensor("sel48", [128, 32], BF, kind="ExternalInput")
    d_bsel = nc.dram_tensor("bsel", [128, 128], BF, kind="ExternalInput")
    d_out = nc.dram_tensor("out", [1024, 384], BF, kind="ExternalOutput")

    EXP = mybir.ActivationFunctionType.Exp

    with tile.TileContext(nc) as tc:
        with tc.tile_pool(name="singles", bufs=1) as S:
            wqk = S.tile([128, 3, 4, 128], BF)
            xTs = [S.tile([128, 1024], BF, name=f"xT{k}") for k in range(3)]
            wv = S.tile([128, 3, 192], BF)
            wp = S.tile([128, 2, 384], BF)
            mask = S.tile([128, 320], BF)
            noob16 = S.tile([1, 1024], BF)
            e2 = S.tile([1, 128], BF)
            sel48 = S.tile([128, 32], BF)
            bsel = S.tile([128, 128], BF)
            qT2 = S.tile([128, 2, 1024], BF)
            kT2 = S.tile([128, 2, 1024], BF)
            vaug = S.tile([128, 8, 4, 64], BF)
            nhat = S.tile([128, 2, 1024], BF)
            recsF = [S.tile([64, 512], F, name=f"recF{i}") for i in range(2)]
            recsB = [S.tile([64, 512], BF, name=f"recB{i}")
                     for i in range(2)]
            oTs = [S.tile([128, 512], BF, name=f"oT{s}") for s in range(4)]

            nc.sync.dma_start(out=wqk[:], in_=d_wqk[:])
            nc.sync.dma_start(out=xTs[0][:], in_=d_xT[0][:])
            for sb, dr in [
                (mask, d_mask), (noob16, d_noob), (e2, d_e2),
                (sel48, d_sel), (bsel, d_bsel),
            ]:
                nc.sync.dma_start(out=sb[:], in_=dr[:])
            nc.sync.dma_start(out=xTs[1][:], in_=d_xT[1][:])
            nc.sync.dma_start(out=xTs[2][:], in_=d_xT[2][:])
            nc.sync.dma_start(out=wv[:], in_=d_wv[:])
            nc.sync.dma_start(out=wp[:], in_=d_wp[:])

            nc.vector.memset(vaug[:, :, :, 49:64], 0.0)
            nc.vector.memset(vaug[:, :, :, 48:49], 1.0)

            # ---- phase 1: q^T/k^T per head-pair, v natural -----------
            with tc.tile_pool(name="psA", bufs=2, space="PSUM") as psA:
                for ti in range(4):  # 2*pr + s; s: 0=q, 1=k
                    pr, s = divmod(ti, 2)
                    dst = qT2 if s == 0 else kT2
                    for hf in range(2):
                        A = psA.tile([128, 512], F, tag="A")
                        for k in range(3):
                            nc.tensor.matmul(
                                A[:], wqk[:, k, ti, :],
                                xTs[k][:, 512 * hf:512 * hf + 512],
                                start=(k == 0), stop=(k == 2))
                        if (ti + hf) % 2 == 0:
                            nc.scalar.copy(
                                dst[:, pr, 512 * hf:512 * hf + 512], A[:])
                        else:
                            nc.vector.tensor_copy(
                                dst[:, pr, 512 * hf:512 * hf + 512], A[:])
                for pt in range(8):
                    V = psA.tile([128, 192], F, tag="V")
                    for k in range(3):
                        nc.tensor.matmul(
                            V[:], xTs[k][:, 128 * pt:128 * pt + 128],
                            wv[:, k, :], start=(k == 0), stop=(k == 2))
                    if pt % 2 == 0:
                        nc.vector.tensor_copy(
                            vaug[:, pt, :, 0:48],
                            V[:].rearrange("p (h d) -> p h d", h=4))
                    else:
                        nc.scalar.copy(
                            vaug[:, pt, :, 0:48],
                            V[:].rearrange("p (h d) -> p h d", h=4))

            # ---- phase 2: scores, exp, mask for all 4 heads ----------
            with tc.tile_pool(name="psS", bufs=2, space="PSUM") as psS, \
                 tc.tile_pool(name="psO", bufs=1, space="PSUM") as psO, \
                 tc.tile_pool(name="psD", bufs=1, space="PSUM") as psD, \
                 tc.tile_pool(name="psB", bufs=1, space="PSUM") as psB, \
                 tc.tile_pool(name="psP", bufs=2, space="PSUM") as psP, \
                 tc.tile_pool(name="sb2", bufs=4) as sb2, \
                 tc.tile_pool(name="sbo", bufs=2) as sbo:
                eTs = []
                for h in range(4):
                    pr, e = divmod(h, 2)
                    eT = sb2.tile([128, 8, 320], BF, tag="eT", name=f"eT{h}")
                    eTs.append(eT)
                    for jt in range(8):
                        qlo, qhi, mo = SPANS8[jt]
                        spn = qhi - qlo
                        ps = psS.tile([128, 320], F, tag="s")
                        nc.tensor.matmul(
                            ps[:, 0:spn],
                            kT2[64 * e:64 * e + 64, pr,
                                128 * jt:128 * jt + 128],
                            qT2[64 * e:64 * e + 64, pr, qlo:qhi],
                            start=True, stop=True)
                        nc.scalar.activation(eT[:, jt, 0:spn],
                                             ps[:, 0:spn], EXP, scale=SCALE)
                        eng = nc.gpsimd if jt in GPS_JT else nc.vector
                        eng.tensor_mul(eT[:, jt, 0:spn], eT[:, jt, 0:spn],
                                       mask[:, mo:mo + spn])

                # ---- phase 3: PV + den gather, both halves -----------
                Ds = []
                for hf in range(2):
                    D = psD.tile([128, 512], F, tag="D", name=f"D{hf}")
                    Ds.append(D)
                    for pr in range(2):
                        s = 2 * pr + hf
                        O = psO.tile([128, 512], F, tag=f"O{pr}",
                                     name=f"O{pr}_{hf}")
                        nc.tensor.matmul(O[:], e2[:],
                                         noob16[:, 512 * hf:512 * hf + 512],
                                         start=True, stop=False,
                                         skip_group_check=True)
                        pv = PV_HALF[hf]
                        for e in range(2):
                            for i, (jt, lo, hi) in enumerate(pv):
                                last = (e == 1) and (i == len(pv) - 1)
                                elo = lo - SPANS8[jt][0]
                                nc.tensor.matmul(
                                    O[64 * e:64 * e + 64,
                                      lo - 512 * hf:hi - 512 * hf],
                                    vaug[:, jt, 2 * pr + e, :],
                                    eTs[2 * pr + e][:, jt, elo:elo + hi - lo],
                                    start=False, stop=last,
                                    skip_group_check=True)
                        if pr == 0:
                            nc.scalar.copy(oTs[s][:], O[:])
                        else:
                            nc.vector.tensor_copy(oTs[s][:], O[:])
                        nc.tensor.matmul(D[32 * pr:32 * pr + 32, :],
                                         sel48[:, 0:32], oTs[s][:],
                                         start=True, stop=True,
                                         skip_group_check=True)
                    nc.vector.reciprocal_approx_fast(recsF[hf][:],
                                                     D[0:64, :])
                    nc.vector.tensor_copy(recsB[hf][:], recsF[hf][:])

                # ---- phases 4-5: normalize + proj per half -----------
                for hf in range(2):
                    for pr in range(2):
                        s = 2 * pr + hf
                        Bc = psB.tile([128, 512], F, tag="Bc",
                                      name=f"Bc{s}")
                        nc.tensor.matmul(Bc[:], bsel[32 * pr:32 * pr + 32, :],
                                         recsB[hf][32 * pr:32 * pr + 32, :],
                                         start=True, stop=True)
                        nc.vector.tensor_mul(
                            nhat[:, pr, 512 * hf:512 * hf + 512],
                            oTs[s][:], Bc[:])
                    for it in range(4 * hf, 4 * hf + 4):
                        P = psP.tile([128, 384], F, tag="P")
                        for pr in range(2):
                            nc.tensor.matmul(
                                P[:], nhat[:, pr, 128 * it:128 * it + 128],
                                wp[:, pr, :], start=(pr == 0),
                                stop=(pr == 1))
                        ot = sbo.tile([128, 384], BF, tag="ot")
                        nc.scalar.copy(ot[:], P[:])
                        nc.sync.dma_start(
                            out=d_out[128 * it:128 * it + 128, :], in_=ot[:])

    nc.compile()
    _CACHE["nc"] = nc
    return nc


def _host_consts():
    if "consts" in _CACHE:
        return _CACHE["consts"]
    mask = np.zeros((128, 320), np.float32)
    for r in range(4):
        for o in range(10):
            if r <= o <= r + 6:
                kx = np.arange(32)[:, None]
                qx = np.arange(32)[None, :]
                mask[32 * r:32 * r + 32, 32 * o:32 * o + 32] = \
                    (np.abs(kx - qx) <= 3).astype(np.float32)
    noob = np.zeros((1, 1024), np.float32)
    for qy in range(32):
        for qx in range(32):
            oy = max(0, 3 - qy) + max(0, qy - 28)
            ox = max(0, 3 - qx) + max(0, qx - 28)
            noob[0, 32 * qy + qx] = 49 - (7 - oy) * (7 - ox)
    e2 = np.zeros((1, 128), np.float32)
    e2[0, 48] = 1.0
    e2[0, 112] = 1.0
    sel48 = np.zeros((128, 32), np.float32)
    sel48[48, 0] = 1.0
    sel48[112, 1] = 1.0
    sel48[48, 2:32] = 1.0  # filler rows stay finite for the reciprocal
    bsel = np.zeros((128, 128), np.float32)
    for pr in range(2):
        bsel[32 * pr, 0:64] = 1.0
        bsel[32 * pr + 1, 64:128] = 1.0
    consts = dict(maskbin=mask.astype(NPBF), noob16=noob.astype(NPBF),
                  e2=e2.astype(NPBF), sel48=sel48.astype(NPBF),
                  bsel=bsel.astype(NPBF))
    _CACHE["consts"] = consts
    return consts


def _host_weights(w_qkv, w_proj, hg):
    wqk = np.zeros((128, 3, 4, 128), np.float32)
    for k in range(3):
        rows = slice(k * 128, (k + 1) * 128)
        for pr in range(2):
            for s in range(2):
                g0 = hg * 4 + 2 * pr
                off = 384 * s
                wqk[:, k, 2 * pr + s, 0:48] = \
                    w_qkv[rows, off + 48 * g0:off + 48 * g0 + 48]
                wqk[:, k, 2 * pr + s, 64:112] = \
                    w_qkv[rows, off + 48 * (g0 + 1):off + 48 * (g0 + 1) + 48]
    wv = np.ascontiguousarray(
        w_qkv[:, 768 + 192 * hg:768 + 192 * hg + 192]
        .reshape(3, 128, 192).transpose(1, 0, 2))
    wp = np.zeros((128, 2, 384), np.float32)
    for pr in range(2):
        g0 = hg * 4 + 2 * pr
        wp[0:48, pr, :] = w_proj[48 * g0:48 * g0 + 48, :]
        wp[64:112, pr, :] = w_proj[48 * (g0 + 1):48 * (g0 + 1) + 48, :]
    return dict(wqk=wqk.astype(NPBF), wv=wv.astype(NPBF),
                wp=wp.astype(NPBF))


def kernel(x, w_qkv, w_proj, b_proj, H=32, W=32):
    global LAST_RESULT
    x = np.asarray(x, np.float32)
    w_qkv = np.asarray(w_qkv, np.float32)
    w_proj = np.asarray(w_proj, np.float32)
    b_proj = np.asarray(b_proj, np.float32)
    assert x.shape == (1, NPOS, C) and int(H) == 32 and int(W) == 32

    nc = _build_nc()
    consts = _host_consts()
    wmaps = [_host_weights(w_qkv, w_proj, hg) for hg in range(2)]

    x4 = x[0].reshape(T, 1024, C)
    in_maps = []
    for c in range(8):
        t, hg = c // 2, c % 2
        xT = x4[t].T.reshape(3, 128, 1024).astype(NPBF)
        m = {f"xT{k}": np.ascontiguousarray(xT[k]) for k in range(3)}
        m.update(wmaps[hg])
        m.update(consts)
        in_maps.append(m)

    trace = bool(int(os.environ.get("TRACE", "0")))
    res = run_bass_kernel_spmd(nc, in_maps, core_ids=list(range(8)),
                               trace=trace)
    LAST_RESULT = res
    outs = []
    for t in range(T):
        p0 = res.results[2 * t]["out"].astype(np.float32)
        p1 = res.results[2 * t + 1]["out"].astype(np.float32)
        outs.append(p0 + p1 + b_proj)
    return np.concatenate(outs, axis=0).reshape(1, NPOS, C)
